# revision 2
# baseline (speedup 1.0000x reference)
"""Trainium2 Bass kernel for CrossAttention (b=4, p=8, n=512, dim=512, 8 heads x 64).

Sharding: 32 independent (b, p) slices, 4 per core across 8 NeuronCores
(data parallel, no collectives). Weights replicated. Inputs pre-transposed
per-slice to [dim, n] bf16 on the host.

Device dataflow per slice (SBUF tiles are [partition, free]):
  - qT/kT = Wq/Wk-blocks^T @ xT; v = xkvT-blocks^T @ Wv (PE), PSUM drained
    by DVE to bf16. v lands strided [jb, h, 65] with a ones column per head
    at position 64.
  - per head: ST[j, i] = kT_h-block^T @ qT_h (PE) -> exp (ACT, scale=1/8)
    -> pt bf16. No max subtraction (scores ~N(0,1)).
  - PV: pv[0:65] += v_ext_jb^T @ pt_jb; row 64 accumulates l = sum_j p
    (the ones column) at zero extra PE cost.
  - normalize: DVE reciprocal of row 64 -> Pool partition_broadcast to 64
    rows -> DVE mul into outT16 (bf16). PE untouched.
  - final: fin[i, f] = outT-blocks^T @ Wo (PE); DVE adds broadcast bias
    during the PSUM->SBUF drain; DMA out per 128-row block.

PE program order is software-pipelined: next-slice projection chunks and
prev-slice final-projection tiles are interleaved between per-head
scores/PV rounds so the tensor engine never idles (keeps the p-state ramp
at max clock) while ACT/DVE/Pool run exp/normalize in the shadow.
"""

from collections import deque
from contextlib import ExitStack

import ml_dtypes
import numpy as np

import concourse.bass as bass
import concourse.tile as tile
from concourse import bacc, mybir
from concourse.bass_utils import run_bass_kernel_spmd

F32 = mybir.dt.float32
BF16 = mybir.dt.bfloat16

HEADS = 8
DH = 64
N = 512
DIM = 512
SCALE = DH**-0.5
S = 4  # (b, p) slices per core
N_CORES = 8


def _build_body(ctx: ExitStack, tc: tile.TileContext, qT, kvT, wq, wk, wv, wo, bo, out):
    nc = tc.nc

    const = ctx.enter_context(tc.tile_pool(name="const", bufs=1))
    xT = ctx.enter_context(tc.tile_pool(name="xT", bufs=4))
    proj = ctx.enter_context(tc.tile_pool(name="proj", bufs=2))
    ptp = ctx.enter_context(tc.tile_pool(name="ptp", bufs=4))
    outTp = ctx.enter_context(tc.tile_pool(name="outTp", bufs=8))
    nrmp = ctx.enter_context(tc.tile_pool(name="nrmp", bufs=3))
    finp = ctx.enter_context(tc.tile_pool(name="finp", bufs=2))
    mm_ps = ctx.enter_context(tc.tile_pool(name="mm_ps", bufs=2, space="PSUM"))
    st_ps = ctx.enter_context(tc.tile_pool(name="st_ps", bufs=4, space="PSUM"))
    pv_ps = ctx.enter_context(tc.tile_pool(name="pv_ps", bufs=2, space="PSUM"))

    # --- weights (bf16 in DRAM): [512, 512] -> [128, 4*512] ---
    # DMA priority order: the first projection chunk needs wq + xqT(0), so
    # those are issued first; everything else queues behind.
    w_sb = {
        name: const.tile([128, 4 * 512], BF16, name=f"{name}16")
        for name in ("wq", "wk", "wv", "wo")
    }
    wq16, wk16, wv16, wo16 = (w_sb[k] for k in ("wq", "wk", "wv", "wo"))

    def dma_w(name, dram, split=False):
        if split:
            # per-contraction-block chunks: the first projection matmul only
            # needs block d=0, so it can start after 128KB instead of 512KB
            for d in range(4):
                nc.sync.dma_start(
                    w_sb[name][:, d * 512 : (d + 1) * 512],
                    dram[d * 128 : (d + 1) * 128, :],
                )
        else:
            nc.sync.dma_start(
                w_sb[name][:], dram.rearrange("(t p) e -> p t e", p=128)
            )

    bo32 = const.tile([1, 512], F32, name="bo32")
    bob = const.tile([128, 512], F32, name="bob")

    # per-slice state
    x_tiles = [None] * (S + 1)
    qkv = [None] * (S + 1)  # (qT16, kT16, v16)
    pt_tiles = {}  # (s, h) -> pt16
    pv_tiles = {}  # (s, h) -> pv psum
    outT = [None] * (S + 1)

    def dma_xq(s, split=False):
        xq = xT.tile([128, 4 * 512], BF16, name="xqT")
        if split:
            for d in range(4):
                nc.sync.dma_start(
                    xq[:, d * 512 : (d + 1) * 512], qT[s][d * 128 : (d + 1) * 128, :]
                )
        else:
            nc.sync.dma_start(xq[:], qT[s].rearrange("(t p) n -> p t n", p=128))
        return xq

    def dma_xkv(s, split=False):
        xkv = xT.tile([128, 4 * 512], BF16, name="xkvT")
        if split:
            for d in range(4):
                nc.sync.dma_start(
                    xkv[:, d * 512 : (d + 1) * 512], kvT[s][d * 128 : (d + 1) * 128, :]
                )
        else:
            nc.sync.dma_start(xkv[:], kvT[s].rearrange("(t p) n -> p t n", p=128))
        return xkv

    def dma_in(s):
        if s >= S:
            return
        x_tiles[s] = (dma_xq(s), dma_xkv(s))

    def proj_chunks(s):
        """Yield 12 callables, each emitting 4 PE matmuls (+1 drain)."""
        if s >= S:
            return
        qT16 = proj.tile([128, 4 * 512], BF16, name="qT16")
        kT16 = proj.tile([128, 4 * 512], BF16, name="kT16")
        v16 = proj.tile([128, 4 * 520], BF16, name="v16")
        # ones columns at jb*520 + h*65 + 64 (survive until overwritten;
        # only this memset ever writes them)
        ones_view = v16[:].rearrange("p (j h o) -> p j h o", j=4, h=8)[:, :, :, 64:65]
        nc.gpsimd.memset(ones_view, 1.0)
        qkv[s] = (qT16, kT16, v16)
        xq, xkv = x_tiles[s]

        def qk_chunk(w16, xt, dst, t):
            def emit():
                ps = mm_ps.tile([128, 512], F32, name="mm_ps")
                for d in range(4):
                    nc.tensor.matmul(
                        ps[:],
                        w16[:, d * 512 + t * 128 : d * 512 + (t + 1) * 128],
                        xt[:, d * 512 : (d + 1) * 512],
                        start=(d == 0),
                        stop=(d == 3),
                    )
                nc.vector.tensor_copy(dst[:, t * 512 : (t + 1) * 512], ps[:])
            return emit

        def v_chunk(jb):
            def emit():
                ps = mm_ps.tile([128, 512], F32, name="mm_ps")
                for d in range(4):
                    nc.tensor.matmul(
                        ps[:],
                        xkv[:, d * 512 + jb * 128 : d * 512 + (jb + 1) * 128],
                        wv16[:, d * 512 : (d + 1) * 512],
                        start=(d == 0),
                        stop=(d == 3),
                    )
                dst = v16[:, jb * 520 : (jb + 1) * 520]
                dst = dst.rearrange("p (h o) -> p h o", h=8)[:, :, 0:64]
                nc.vector.tensor_copy(dst, ps[:].rearrange("p (h o) -> p h o", h=8))
            return emit

        for t in range(4):
            yield qk_chunk(wq16, xq, qT16, t)
            yield qk_chunk(wk16, xkv, kT16, t)
            yield v_chunk(t)

    def scores(s, h):
        qT16, kT16, _ = qkv[s]
        tp, half = h // 2, (h % 2) * 64
        kT_h = kT16[half : half + 64, tp * 512 : (tp + 1) * 512]
        qT_h = qT16[half : half + 64, tp * 512 : (tp + 1) * 512]
        pt16 = ptp.tile([128, 4 * 512], BF16, name="pt16")
        for jb in range(4):
            stt = st_ps.tile([128, 512], F32, name="st_ps")
            nc.tensor.matmul(
                stt[:], kT_h[:, jb * 128 : (jb + 1) * 128], qT_h,
                start=True, stop=True,
            )
            nc.scalar.activation(
                pt16[:, jb * 512 : (jb + 1) * 512],
                stt[:],
                mybir.ActivationFunctionType.Exp,
                scale=SCALE,
            )
        pt_tiles[(s, h)] = pt16

    def pv(s, h):
        _, _, v16 = qkv[s]
        pt16 = pt_tiles.pop((s, h))
        pvt = pv_ps.tile([128, 512], F32, name="pv_ps")
        for jb in range(4):
            nc.tensor.matmul(
                pvt[0:65, :],
                v16[:, jb * 520 + h * 65 : jb * 520 + (h + 1) * 65],
                pt16[:, jb * 512 : (jb + 1) * 512],
                start=(jb == 0),
                stop=(jb == 3),
            )
        pv_tiles[(s, h)] = pvt

    def normalize(s, h):
        tp, half = h // 2, (h % 2) * 64
        if outT[s] is None:
            # one tile per head pair: the final projection's t-block matmul
            # then only depends on that pair's two normalizes, not all 8
            outT[s] = [
                outTp.tile([128, 512], BF16, name=f"outT16_{t}") for t in range(4)
            ]
        pvt = pv_tiles.pop((s, h))
        rinv = nrmp.tile([1, 512], F32, name="rinv")
        nc.vector.reciprocal(rinv[:], pvt[64:65, :])
        rb = nrmp.tile([64, 512], F32, name="rb")
        nc.gpsimd.partition_broadcast(rb[:], rinv[:])
        nc.vector.tensor_mul(
            outT[s][tp][half : half + 64, :],
            pvt[0:64, :],
            rb[:],
        )

    def final_tiles(s):
        """Yield 4 callables, each emitting one fin psum tile (4 mm + drain)."""
        outT16 = outT[s]

        def fin_chunk(ib):
            def emit():
                ps = mm_ps.tile([128, 512], F32, name="mm_ps")
                for t in range(4):
                    nc.tensor.matmul(
                        ps[:],
                        outT16[t][:, ib * 128 : (ib + 1) * 128],
                        wo16[:, t * 512 : (t + 1) * 512],
                        start=(t == 0),
                        stop=(t == 3),
                    )
                fin = finp.tile([128, 512], F32, name="fin")
                nc.vector.tensor_add(fin[:], ps[:], bob[:])
                nc.sync.dma_start(out[s][ib * 128 : (ib + 1) * 128, :], fin[:])
            return emit

        for ib in range(4):
            yield fin_chunk(ib)

    # ---- emission ----
    # PE warmup first (no DMA deps): dummy matmuls on memset data while the
    # first DMAs land — the p-state ramp completes before the first real
    # matmul and PE continuity bridges straight into the prologue.
    warm = const.tile([128, 512], BF16, name="warm")
    nc.gpsimd.memset(warm[:], 1.0)
    for _ in range(10):
        wps = mm_ps.tile([128, 512], F32, name="mm_ps")
        nc.tensor.matmul(wps[:], warm[:, 0:128], warm[:], start=True, stop=True)

    # Startup DMA priority: first projection chunk needs wq + xqT(0); SP
    # dispatches serialize at ~650ns each, so order is completion order.
    dma_w("wq", wq)
    xq0 = dma_xq(0)
    dma_w("wk", wk)
    xkv0 = dma_xkv(0)
    dma_w("wv", wv)
    dma_w("wo", wo)
    x_tiles[0] = (xq0, xkv0)

    def split_chunks(s):
        """12 proj chunks -> (early8 emitted a slice ahead, own4 kept for
        slice s's own rounds r0-r1; t2/t3 q/k blocks are only needed from
        head 4 on)."""
        c = list(proj_chunks(s))
        if not c:
            return [], []
        # arrival-order for the slice-0 prologue: q chunks need only wq+xq
        early8 = [c[0], c[3], c[1], c[4], c[2], c[5], c[8], c[11]]
        own4 = [c[6], c[7], c[9], c[10]]
        return early8, own4

    early8, own4 = split_chunks(0)
    for chunk in early8:  # prologue, not interleaved
        chunk()
    dma_in(1)
    # bias is first needed by fin(0) drains ~40us in; keep its DMA out of
    # the startup critical path
    nc.sync.dma_start(bo32[:], bo.rearrange("(o f) -> o f", o=1))
    nc.gpsimd.partition_broadcast(bob[:], bo32[:])
    own4_next = own4

    for s in range(S):
        early8, own4_future = split_chunks(s + 1)
        fin4 = list(final_tiles(s - 1)) if s >= 1 else []
        # round plan (2 slots/round): r0,r1 own4 | r2 early q/k | r3,r4 fin
        # (gives normalize(s-1,7) slack) | r5..r7 rest of early8
        fillers = deque(own4_next + early8[:2] + fin4 + early8[2:])
        own4_next = own4_future
        for h in range(HEADS):
            for _ in range(2):
                if fillers:
                    fillers.popleft()()
            scores(s, h)
            if h >= 1:
                pv(s, h - 1)
                normalize(s, h - 1)
            if h == 4:
                dma_in(s + 2)
        while fillers:
            fillers.popleft()()
        pv(s, 7)
        normalize(s, 7)
    for chunk in final_tiles(S - 1):
        chunk()


def build_nc():
    nc = bacc.Bacc("TRN2", target_bir_lowering=False, debug=False)
    qT = nc.dram_tensor("qT", [S, DIM, N], BF16, kind="ExternalInput").ap()
    kvT = nc.dram_tensor("kvT", [S, DIM, N], BF16, kind="ExternalInput").ap()
    wq = nc.dram_tensor("wq", [DIM, DIM], BF16, kind="ExternalInput").ap()
    wk = nc.dram_tensor("wk", [DIM, DIM], BF16, kind="ExternalInput").ap()
    wv = nc.dram_tensor("wv", [DIM, DIM], BF16, kind="ExternalInput").ap()
    wo = nc.dram_tensor("wo", [DIM, DIM], BF16, kind="ExternalInput").ap()
    bo = nc.dram_tensor("bo", [DIM], F32, kind="ExternalInput").ap()
    out = nc.dram_tensor("out", [S, N, DIM], F32, kind="ExternalOutput").ap()
    with tile.TileContext(nc) as tc:
        with ExitStack() as ctx:
            _build_body(ctx, tc, qT, kvT, wq, wk, wv, wo, bo, out)
    nc.compile()
    return nc


_NC = None
BF = ml_dtypes.bfloat16


def make_in_maps(q_in, kv_in, Wq, Wk, Wv, Wo, bo):
    # host-side layout prep: per-slice transpose to [dim, n] + bf16 cast
    q = np.asarray(q_in, dtype=np.float32).reshape(32, N, DIM)
    kv = np.asarray(kv_in, dtype=np.float32).reshape(32, N, DIM)
    qT = np.ascontiguousarray(q.transpose(0, 2, 1)).astype(BF)
    kvT = np.ascontiguousarray(kv.transpose(0, 2, 1)).astype(BF)
    w = {
        "wq": np.asarray(Wq, dtype=np.float32).astype(BF),
        "wk": np.asarray(Wk, dtype=np.float32).astype(BF),
        "wv": np.asarray(Wv, dtype=np.float32).astype(BF),
        "wo": np.asarray(Wo, dtype=np.float32).astype(BF),
        "bo": np.asarray(bo, dtype=np.float32),
    }
    return [
        {"qT": qT[S * c : S * (c + 1)], "kvT": kvT[S * c : S * (c + 1)], **w}
        for c in range(N_CORES)
    ]


def kernel(q_in, kv_in, Wq, Wk, Wv, Wo, bo):
    global _NC
    if _NC is None:
        _NC = build_nc()
    in_maps = make_in_maps(q_in, kv_in, Wq, Wk, Wv, Wo, bo)
    res = run_bass_kernel_spmd(_NC, in_maps, list(range(N_CORES))).results
    out = np.concatenate([res[c]["out"] for c in range(N_CORES)], axis=0)
    return out.reshape(4, 8, N, DIM)
